# revision 1
# baseline (speedup 1.0000x reference)
"""Trainium2 Bass kernel for nn_ContextualEncoder2 (5-step GRU over buoys).

Strategy (data-parallel over 16384 buoys across 8 cores, 2048 each):
  * Transposed compute layout: gate-features on SBUF partitions, buoys on
    the free axis. h stays [H, cols] in SBUF between steps -> no transposes.
  * gates.T = W.T-tiles (stationary, fp32r) @ h.T-tiles (moving, fp32r),
    accumulated in PSUM [128, 512] tiles. All contractions are K=128
    (obs/onehot operands are zero-padded on host so no slow partial-row
    matmuls appear in the PE stream).
  * The embedding gather emb[ids] @ W_ih[:,64:].T is algebraically replaced
    by a onehot(ids) matmul against emb_proj = emb @ W_ih[:,64:].T (one
    extra K=128 matmul per PSUM tile). emb_proj is computed on device once.
  * Step 5 uses W_hh + W_ih[:, :1024] summed on host (its gi and gh parts
    both consume h4), saving one full contraction for the r/z gates.
  * All biases are applied as per-partition ACT bias operands.
  * outs[0] (h after step 1) is spilled to DRAM and streamed back in step 4.
  * Block schedule is software-pipelined: block b+1's latency-bound step 1
    is emitted before block b's step 5 so its chains hide under PE work.
"""
import numpy as np

import concourse.bass as bass
import concourse.mybir as mybir
import concourse.tile as tile
from concourse import bacc
from concourse.bass_utils import run_bass_kernel_spmd

F32 = mybir.dt.float32
F32R = mybir.dt.float32r
AF = mybir.ActivationFunctionType
OP = mybir.AluOpType

N_CORES = 8
NUM_BUOYS = 16384
H = 1024
G3 = 3072
NEMB = 100
KCH = 8          # 1024 / 128 contraction chunks
FCH = 8          # 1024 / 128 gate-feature tiles
NT = 512         # moving/free tile width (one PSUM bank of fp32)
NS = G3 // NT    # emb_proj column slices


def _accum(nc, psum, pairs):
    last = len(pairs) - 1
    for i, (l, r) in enumerate(pairs):
        nc.tensor.matmul(psum, l, r, start=(i == 0), stop=(i == last))


def build(nbuoy=2048, blk=1024):
    """Build the per-core Bass program (same NEFF on every core)."""
    assert nbuoy % blk == 0 and blk % NT == 0
    nblk = nbuoy // blk
    J = blk // NT

    nc = bacc.Bacc("TRN2", target_bir_lowering=False, debug=False)

    whh = nc.declare_dram_parameter("whh", [24, 128, 1024], F32R, isOutput=False)
    wih = nc.declare_dram_parameter("wih", [24, 128, 1024], F32R, isOutput=False)
    wsum = nc.declare_dram_parameter("wsum", [16, 128, 1024], F32R, isOutput=False)
    wemb = nc.declare_dram_parameter("wemb", [KCH, 128, G3], F32R, isOutput=False)
    wobs = nc.declare_dram_parameter("wobs", [128, G3], F32R, isOutput=False)
    embt = nc.declare_dram_parameter("embt", [KCH, 128, NEMB], F32R, isOutput=False)
    # onehot rows 100..127 are zero; obs tiles carry a zero half (see prep)
    onehot = nc.declare_dram_parameter("onehot", [128, nbuoy], F32R, isOutput=False)
    obs0 = nc.declare_dram_parameter("obs0", [128, nbuoy], F32R, isOutput=False)
    obs23 = nc.declare_dram_parameter("obs23", [2, 128, nbuoy], F32R, isOutput=False)
    obs45 = nc.declare_dram_parameter("obs45", [2, 128, nbuoy], F32R, isOutput=False)
    bih = nc.declare_dram_parameter("bih", [128, 24], F32, isOutput=False)
    bhh = nc.declare_dram_parameter("bhh", [128, 24], F32, isOutput=False)
    out_t = nc.declare_dram_parameter("out_t", [FCH, 128, nbuoy], F32, isOutput=True)

    whh_ap, wih_ap, wsum_ap, wemb_ap = whh.ap(), wih.ap(), wsum.ap(), wemb.ap()
    out_ap = out_t.ap()
    obs23_ap, obs45_ap = obs23.ap(), obs45.ap()

    with tile.TileContext(nc) as tc:
        with (
            tc.tile_pool(name="const", bufs=1) as cpool,
            tc.tile_pool(name="obsl", bufs=1) as opool,
            tc.tile_pool(name="htiles", bufs=1) as hpool,
            tc.tile_pool(name="work", bufs=2) as wpool,
        ):
            emb_proj = cpool.tile([128, G3], F32R, tag="embproj")
            nc.gpsimd.memset(emb_proj[96:128, :].bitcast(F32), 0.0)

            # ---- phase A: emb_proj[:100] = emb @ W_ih[:, 64:1088].T --------
            # (emitted before the constant loads so its 12.6MB weight stream
            #  heads the DMA queues -- everything else waits on it anyway)
            with (
                tc.tile_pool(name="phA", bufs=2) as apool,
                tc.tile_pool(name="psA", bufs=1, space="PSUM") as psA,
            ):
                with nc.named_scope("embproj"):
                    emb_sb = apool.tile([128, KCH * NEMB], F32R, tag="embt")
                    for k in range(KCH):
                        nc.sync.dma_start(
                            emb_sb[:, k * NEMB:(k + 1) * NEMB], embt.ap()[k])
                    psums = [psA.tile([NEMB, NT], F32, tag=f"embp{s}",
                                      name=f"embp{s}") for s in range(NS)]
                    for k in range(KCH):
                        wk = apool.tile([128, G3], F32R, tag="wemb", name="wk")
                        nc.sync.dma_start(wk[:], wemb_ap[k])
                        for s in range(NS):
                            nc.tensor.matmul(
                                psums[s][:],
                                emb_sb[:, k * NEMB:(k + 1) * NEMB],
                                wk[:, s * NT:(s + 1) * NT],
                                start=(k == 0), stop=(k == KCH - 1))
                    for s in range(NS):
                        nc.scalar.activation(
                            emb_proj[:NEMB, s * NT:(s + 1) * NT], psums[s][:],
                            AF.Copy)

            # ---- constants -------------------------------------------------
            bih_sb = cpool.tile([128, 24], F32, tag="bih")
            nc.sync.dma_start(bih_sb[:], bih.ap())
            bhh_sb = cpool.tile([128, 24], F32, tag="bhh")
            nc.sync.dma_start(bhh_sb[:], bhh.ap())
            bsum = cpool.tile([128, 24], F32, tag="bsum")
            nc.vector.tensor_add(bsum[:], bih_sb[:], bhh_sb[:])
            wobs_sb = cpool.tile([128, G3], F32R, tag="wobs")
            nc.sync.dma_start(wobs_sb[:], wobs.ap())

            # ---- phase B ---------------------------------------------------
            with (
                tc.tile_pool(name="wstr", bufs=8) as spool,
                tc.tile_pool(name="psB", bufs=2, space="PSUM") as psB,
            ):
                _rr = [0]
                _tags = ["pr", "pz", "pgh", "pg"]

                def rrtile():
                    t = psB.tile([128, NT], F32, tag=_tags[_rr[0] % 4],
                                 name=f"rr{_rr[0]}")
                    _rr[0] += 1
                    return t

                st = {b: {"h": {}, "h1": {}, "obs": {}} for b in range(nblk)}

                def init_block(b):
                    cb = b * blk
                    o = st[b]["obs"]
                    t = opool.tile([128, blk], F32R, tag="obs0", name="obs0t")
                    nc.sync.dma_start(t[:], obs0.ap()[:, cb:cb + blk])
                    o[1] = t
                    for i, (ap_, tagp) in enumerate(
                            [(obs23_ap, "o23"), (obs45_ap, "o45")]):
                        for s2 in range(2):
                            t = opool.tile([128, blk], F32R,
                                           tag=f"{tagp}_{s2}", name=f"{tagp}{s2}")
                            nc.sync.dma_start(t[:], ap_[s2][:, cb:cb + blk])
                            o[2 + i * 2 + s2] = t   # keys 2,3 (s2/s3), 4,5
                    t = opool.tile([128, blk], F32R, tag="oh", name="oht")
                    nc.sync.dma_start(t[:], onehot.ap()[:, cb:cb + blk])
                    o["oh"] = t

                def step1(b):
                    cb, h, obs = b * blk, st[b]["h"], st[b]["obs"]
                    with nc.named_scope(f"b{b}s1"):
                        for jj in range(J):
                            for f in range(FCH):
                                c0, c1 = jj * NT, (jj + 1) * NT
                                mr, mz, mn = f, 8 + f, 16 + f
                                ps = {}
                                for m, key in ((mr, "r"), (mz, "z"), (mn, "g")):
                                    p = rrtile()
                                    _accum(nc, p[:], [
                                        (wobs_sb[:, m * 128:(m + 1) * 128],
                                         obs[1][:, c0:c1]),
                                        (emb_proj[:, m * 128:(m + 1) * 128],
                                         obs["oh"][:, c0:c1])])
                                    ps[key] = p
                                r = wpool.tile([128, NT], F32, tag="r", name="r")
                                nc.scalar.activation(r[:], ps["r"][:], AF.Sigmoid,
                                                     bias=bsum[:, mr:mr + 1])
                                z = wpool.tile([128, NT], F32, tag="z", name="z")
                                nc.scalar.activation(z[:], ps["z"][:], AF.Sigmoid,
                                                     bias=bsum[:, mz:mz + 1])
                                t2 = wpool.tile([128, NT], F32, tag="t2", name="t2")
                                nc.vector.scalar_tensor_tensor(
                                    t2[:], r[:], bhh_sb[:, mn:mn + 1], ps["g"][:],
                                    OP.mult, OP.add)
                                n_t = wpool.tile([128, NT], F32, tag="n", name="n")
                                nc.scalar.activation(n_t[:], t2[:], AF.Tanh,
                                                     bias=bih_sb[:, mn:mn + 1])
                                v = wpool.tile([128, NT], F32, tag="tmp", name="v")
                                nc.vector.tensor_mul(v[:], z[:], n_t[:])
                                hn = hpool.tile([128, NT], F32R,
                                                tag=f"hA_{f}_{jj}", name="hn")
                                nc.vector.tensor_sub(hn[:], n_t[:], v[:])
                                h[(f, jj)] = hn
                                st[b]["h1"][(f, jj)] = hn

                def stepn(b, s):
                    cb, h, obs = b * blk, st[b]["h"], st[b]["obs"]
                    fam = {2: "hB", 3: "hC", 4: "hB"}.get(s)
                    ot = obs[s]
                    h1t = st[b]["h1"]
                    hnew = {}
                    with nc.named_scope(f"b{b}s{s}"):
                        for f in range(FCH):
                            mr, mz, mn = f, 8 + f, 16 + f

                            def wsl(ap_, m):
                                t = spool.tile([128, 1024], F32R, tag="wsl",
                                               name="wsl")
                                nc.sync.dma_start(t[:], ap_[m])
                                return t

                            wr = wsl(wsum_ap if s == 5 else whh_ap, mr)
                            wz = wsl(wsum_ap if s == 5 else whh_ap, mz)
                            wn = wsl(whh_ap, mn)
                            if s == 4:
                                vr, vz = wsl(wih_ap, mr), wsl(wih_ap, mz)
                            if s >= 4:
                                vn = wsl(wih_ap, mn)
                            for jj in range(J):
                                c0, c1 = jj * NT, (jj + 1) * NT
                                hcol = [h[(k, jj)] for k in range(KCH)]
                                gcol = ([h1t[(k, jj)] for k in range(KCH)]
                                        if s == 4 else hcol)

                                def wmm(w, col):
                                    return [(w[:, k * 128:(k + 1) * 128],
                                             col[k][:]) for k in range(KCH)]

                                pr = psB.tile([128, NT], F32, tag="pr")
                                pairs = wmm(wr, hcol)
                                if s == 4:
                                    pairs += wmm(vr, gcol)
                                pairs.append((wobs_sb[:, mr * 128:(mr + 1) * 128],
                                              ot[:, c0:c1]))
                                if s <= 3:
                                    pairs.append(
                                        (emb_proj[:, mr * 128:(mr + 1) * 128],
                                         obs["oh"][:, c0:c1]))
                                _accum(nc, pr[:], pairs)

                                pz = psB.tile([128, NT], F32, tag="pz")
                                pairs = wmm(wz, hcol)
                                if s == 4:
                                    pairs += wmm(vz, gcol)
                                pairs.append((wobs_sb[:, mz * 128:(mz + 1) * 128],
                                              ot[:, c0:c1]))
                                if s <= 3:
                                    pairs.append(
                                        (emb_proj[:, mz * 128:(mz + 1) * 128],
                                         obs["oh"][:, c0:c1]))
                                _accum(nc, pz[:], pairs)

                                pgh = psB.tile([128, NT], F32, tag="pgh")
                                _accum(nc, pgh[:], wmm(wn, hcol))

                                pg = psB.tile([128, NT], F32, tag="pg")
                                if s <= 3:
                                    pairs = [
                                        (wobs_sb[:, mn * 128:(mn + 1) * 128],
                                         ot[:, c0:c1]),
                                        (emb_proj[:, mn * 128:(mn + 1) * 128],
                                         obs["oh"][:, c0:c1])]
                                else:
                                    pairs = wmm(vn, gcol)
                                    pairs.append(
                                        (wobs_sb[:, mn * 128:(mn + 1) * 128],
                                         ot[:, c0:c1]))
                                _accum(nc, pg[:], pairs)

                                r = wpool.tile([128, NT], F32, tag="r", name="r")
                                nc.scalar.activation(r[:], pr[:], AF.Sigmoid,
                                                     bias=bsum[:, mr:mr + 1])
                                z = wpool.tile([128, NT], F32, tag="z", name="z")
                                nc.scalar.activation(z[:], pz[:], AF.Sigmoid,
                                                     bias=bsum[:, mz:mz + 1])
                                # t1 = (gh_n + b_hh_n) * r
                                t1 = wpool.tile([128, NT], F32, tag="tmp",
                                                name="t1")
                                nc.vector.scalar_tensor_tensor(
                                    t1[:], pgh[:], bhh_sb[:, mn:mn + 1], r[:],
                                    OP.add, OP.mult)
                                t2 = wpool.tile([128, NT], F32, tag="t2",
                                                name="t2")
                                nc.vector.tensor_add(t2[:], t1[:], pg[:])
                                n_t = wpool.tile([128, NT], F32, tag="n", name="n")
                                nc.scalar.activation(n_t[:], t2[:], AF.Tanh,
                                                     bias=bih_sb[:, mn:mn + 1])
                                d = wpool.tile([128, NT], F32, tag="tmp2",
                                               name="d")
                                nc.vector.tensor_sub(
                                    d[:], h[(f, jj)][:].bitcast(F32), n_t[:])
                                e = wpool.tile([128, NT], F32, tag="tmp", name="e")
                                nc.vector.tensor_mul(e[:], z[:], d[:])
                                if s < 5:
                                    hn = hpool.tile([128, NT], F32R,
                                                    tag=f"{fam}_{f}_{jj}",
                                                    name="hn")
                                    nc.vector.tensor_add(hn[:], n_t[:], e[:])
                                    hnew[(f, jj)] = hn
                                else:
                                    ho = wpool.tile([128, NT], F32, tag="hout",
                                                    name="ho")
                                    nc.vector.tensor_add(ho[:], n_t[:], e[:])
                                    nc.sync.dma_start(
                                        out_ap[f][:, cb + c0:cb + c1], ho[:])
                    if s < 5:
                        st[b]["h"] = hnew

                # software-pipelined block schedule
                sched = [(0, 0), (0, 1)]
                for b in range(nblk):
                    sched += [(b, s) for s in (2, 3, 4)]
                    if b + 1 < nblk:
                        sched += [(b + 1, 0), (b + 1, 1)]
                    sched.append((b, 5))

                for b, s in sched:
                    if s == 0:
                        init_block(b)
                    elif s == 1:
                        step1(b)
                    else:
                        stepn(b, s)

    nc.compile()
    return nc


# ---------------------------------------------------------------------------
# host-side prep / sharding
# ---------------------------------------------------------------------------

def _prep_shared(emb, W_ih, W_hh, b_ih, b_hh):
    f = np.float32
    W_ih = np.asarray(W_ih, f)
    W_hh = np.asarray(W_hh, f)

    def slabs(W):  # (3072, 1024) -> [24, 128, 1024]: [m, i, k*128+j] = W[128m+j, 128k+i]
        t = W.reshape(24, 128, 8, 128)          # [m, j, k, i]
        return np.ascontiguousarray(t.transpose(0, 3, 2, 1).reshape(24, 128, 1024))

    whh = slabs(W_hh)
    wih = slabs(W_ih[:, :1024])
    wsum = np.ascontiguousarray(slabs(W_hh + W_ih[:, :1024])[:16])
    # [k, i, n] = W_ih[n, 64 + 128k + i]
    wemb = np.ascontiguousarray(
        W_ih[:, 64:1088].reshape(G3, 8, 128).transpose(1, 2, 0))
    wobs = np.concatenate(
        [W_ih[:, :64].T, W_ih[:, 1024:1088].T], axis=0)  # [128, 3072]
    wobs = np.ascontiguousarray(wobs, f)
    embt = np.ascontiguousarray(np.asarray(emb, f).T.reshape(8, 128, NEMB))
    bih_t = np.ascontiguousarray(np.asarray(b_ih, f).reshape(24, 128).T)
    bhh_t = np.ascontiguousarray(np.asarray(b_hh, f).reshape(24, 128).T)
    return dict(whh=whh, wih=wih, wsum=wsum, wemb=wemb, wobs=wobs, embt=embt,
                bih=bih_t, bhh=bhh_t)


def _prep_core(buoy_obs, buoy_ids, nbuoy):
    f = np.float32
    o = np.asarray(buoy_obs, f)
    ids = np.asarray(buoy_ids)
    # steps 1-3 hit wobs rows 0:64 -> obs in rows 0:64, zeros in 64:128
    # steps 4-5 hit wobs rows 64:128 -> zeros in 0:64, obs in 64:128
    obs0 = np.zeros((128, nbuoy), f)
    obs0[:64] = o[:, 0, :].T
    obs23 = np.zeros((2, 128, nbuoy), f)
    obs45 = np.zeros((2, 128, nbuoy), f)
    for s in range(2):
        obs23[s, :64] = o[:, s + 1, :].T
        obs45[s, 64:] = o[:, s + 1, :].T
    onehot = np.zeros((128, nbuoy), f)
    onehot[ids, np.arange(nbuoy)] = 1.0
    return dict(obs0=obs0, obs23=obs23, obs45=obs45, onehot=onehot)


_NC_CACHE = {}


def _get_nc(nbuoy, blk):
    key = (nbuoy, blk)
    if key not in _NC_CACHE:
        _NC_CACHE[key] = build(nbuoy, blk)
    return _NC_CACHE[key]


def kernel(buoy_obs, buoy_ids, emb, W_ih, W_hh, b_ih, b_hh):
    buoy_obs = np.asarray(buoy_obs)
    buoy_ids = np.asarray(buoy_ids)
    n = buoy_obs.shape[0]
    per = n // N_CORES
    shared = _prep_shared(emb, W_ih, W_hh, b_ih, b_hh)
    in_maps = []
    for c in range(N_CORES):
        sl = slice(c * per, (c + 1) * per)
        m = dict(shared)
        m.update(_prep_core(buoy_obs[sl], buoy_ids[sl], per))
        in_maps.append(m)

    nc = _get_nc(per, 1024)
    res = run_bass_kernel_spmd(nc, in_maps, list(range(N_CORES)))
    outs = []
    for c in range(N_CORES):
        r = res.results[c]["out_t"]                    # [8, 128, per]
        outs.append(r.transpose(2, 0, 1).reshape(per, H))
    full = np.concatenate(outs, axis=0).astype(np.float32)
    return full[None, :, :]



# revision 11
# speedup vs baseline: 1.4031x; 1.4031x over previous
"""Trainium2 Bass kernel for nn_ContextualEncoder2 (5-step GRU over buoys).

fp8 DoubleRow formulation (data-parallel, 2048 buoys/core, blocks of 1024):

  h_t = hbar_t(id) + d_t  with hbar_t the obs=0 GRU trajectory of the id's
  embedding (100 ids, host-precomputed). All W.h contractions become
    W.h = [W.hbar](id)  (exact per-id table, applied via a onehot matmul)
        + Q8(W).Q8(d)   (residual d is ~10x smaller than h, so 1-term fp8
                         quantization error is ~10x smaller too)
  Every matmul is a float8e4 DoubleRow (2 contraction rows/cycle):
    * tables: stationary [A|B] (2-term fp8 split of 16*T) x moving [oh|oh]
      (onehot columns scaled by 16; PSUM accumulates 256x the math value)
    * obs: stationary [[A;A] | [B;B]] x moving [opack|opack] where opack
      stacks fp8 hi/lo parts of 16*obs on partitions 0:64 / 64:128 - a full
      (A+B)(hi+lo) product in ONE DoubleRow matmul
    * d: stationary fp8(16W) chunk pairs x moving [d_2c|d_2c+1], d = fp8(16d)
  Gate math (1/256 scale folded into ACT), h kept as 16*h in fp16 for the
  elementwise ops; d produced directly in fp8 by a DVE STT reading the
  -256*hbar broadcast PSUM (bc tables applied per position).
"""
import numpy as np
import ml_dtypes

import concourse.bass as bass
import concourse.mybir as mybir
import concourse.tile as tile
from concourse import bacc
from concourse.bass_utils import run_bass_kernel_spmd

F32 = mybir.dt.float32
F16 = mybir.dt.float16
F8 = mybir.dt.float8e4
E4M3 = ml_dtypes.float8_e4m3
AF = mybir.ActivationFunctionType
OP = mybir.AluOpType
DR = mybir.MatmulPerfMode.DoubleRow

N_CORES = 8
NUM_BUOYS = 16384
H = 1024
NEMB = 100
FCH = 8          # 1024/128 gate-feature tiles per gate
KP = 4           # d contraction pairs (1024 / 256)
NT = 512         # moving free tile width (one PSUM bank)


def build(nbuoy=2048, blk=1024):
    assert nbuoy % blk == 0 and blk % NT == 0
    nblk = nbuoy // blk
    J = blk // NT

    nc = bacc.Bacc("TRN2", target_bir_lowering=False, debug=False)

    # --- DRAM parameters -------------------------------------------------
    # d-part weights: [m, p, c, i, j] = fp8(16*W)[128m+j, 128(2c+i)+p]
    whh = nc.declare_dram_parameter("whh", [24, 128, 4, 2, 128], F8, isOutput=False)
    wih = nc.declare_dram_parameter("wih", [24, 128, 4, 2, 128], F8, isOutput=False)
    wsum = nc.declare_dram_parameter("wsum", [16, 128, 4, 2, 128], F8,
                                     isOutput=False)
    # obs stationaries ([A;A] | [B;B] packs)
    wobs = nc.declare_dram_parameter("wobs", [24, 128, 2, 128], F8, isOutput=False)
    wobs45 = nc.declare_dram_parameter("wobs45", [24, 128, 2, 128], F8,
                                       isOutput=False)
    # tables: [tile, p(=id), slot(A|B), j]
    tabs = {}
    for name, nt_ in [("ts1", 24), ("t2rz", 16), ("t2n", 8), ("t3rz", 16),
                      ("t3n", 8), ("t4rz", 16), ("t4n", 8), ("t4gi", 8),
                      ("t5rz", 16), ("t5n", 8), ("t5gi", 8)]:
        tabs[name] = nc.declare_dram_parameter(name, [nt_, 128, 2, 128], F8,
                                               isOutput=False)
    bc = nc.declare_dram_parameter("bc", [4, 8, 128, 2, 128], F8, isOutput=False)
    opk = nc.declare_dram_parameter("opk", [3, 128, nbuoy], F8, isOutput=False)
    oh = nc.declare_dram_parameter("oh", [128, nbuoy], F8, isOutput=False)
    bsum = nc.declare_dram_parameter("bsum", [128, 24], F32, isOutput=False)
    bih = nc.declare_dram_parameter("bih", [128, 24], F32, isOutput=False)
    bhh256 = nc.declare_dram_parameter("bhh256", [128, 24], F32, isOutput=False)
    out_t = nc.declare_dram_parameter("out_t", [FCH, 128, nbuoy], F32,
                                      isOutput=True)

    whh_ap, wih_ap, wsum_ap = whh.ap(), wih.ap(), wsum.ap()
    out_ap = out_t.ap()

    with tile.TileContext(nc) as tc:
        with (
            tc.tile_pool(name="const", bufs=1) as cpool,
            tc.tile_pool(name="obsl", bufs=2) as opool,
            tc.tile_pool(name="htiles", bufs=1) as hpool,
            tc.tile_pool(name="work", bufs=2) as wpool,
            tc.tile_pool(name="wstr", bufs=8) as spool,
            tc.tile_pool(name="psB", bufs=2, space="PSUM") as psB,
        ):
            # ---- resident constants -----------------------------------
            def ctile(shape, tag):
                return cpool.tile(shape, F8, tag=tag, name=tag)

            whh_sb = []
            for m in range(24):
                t = ctile([128, 4, 2, 128], f"whh{m}")
                nc.sync.dma_start(t[:], whh_ap[m])
                whh_sb.append(t)

            def load_tab(name, nt_):
                ts = []
                ap = tabs[name].ap()
                for i in range(nt_):
                    t = ctile([128, 2, 128], f"{name}{i}")
                    nc.sync.dma_start(t[:], ap[i])
                    ts.append(t)
                return ts

            ts1_sb = load_tab("ts1", 24)
            t2rz_sb = load_tab("t2rz", 16)
            t2n_sb = load_tab("t2n", 8)
            t3rz_sb = load_tab("t3rz", 16)
            t3n_sb = load_tab("t3n", 8)
            t4rz_sb = load_tab("t4rz", 16)
            t4n_sb = load_tab("t4n", 8)
            t4gi_sb = load_tab("t4gi", 8)
            t5rz_sb = load_tab("t5rz", 16)
            t5n_sb = load_tab("t5n", 8)
            t5gi_sb = load_tab("t5gi", 8)

            wobs_sb, wobs45_sb = [], []
            for m in range(24):
                t = ctile([128, 2, 128], f"wo{m}")
                nc.sync.dma_start(t[:], wobs.ap()[m])
                wobs_sb.append(t)
                t = ctile([128, 2, 128], f"wo45_{m}")
                nc.sync.dma_start(t[:], wobs45.ap()[m])
                wobs45_sb.append(t)
            bc_sb = [[None] * 8 for _ in range(4)]
            for s in range(4):
                for c in range(8):
                    t = ctile([128, 2, 128], f"bc{s}_{c}")
                    nc.sync.dma_start(t[:], bc.ap()[s][c])
                    bc_sb[s][c] = t

            bsum_sb = cpool.tile([128, 24], F32, tag="bsum", name="bsum")
            nc.sync.dma_start(bsum_sb[:], bsum.ap())
            bih_sb = cpool.tile([128, 24], F32, tag="bih", name="bih")
            nc.sync.dma_start(bih_sb[:], bih.ap())
            bhh_sb = cpool.tile([128, 24], F32, tag="bhh256", name="bhh256")
            nc.sync.dma_start(bhh_sb[:], bhh256.ap())

            # ---- per-block state --------------------------------------
            st = {b: {"h": {}, "obs": {}, "d": {}, "d1": {}} for b in range(nblk)}
            ps_tags = {"pr": "pr", "pz": "pz", "pgh": "pgh", "pg": "pg"}

            def init_block(b):
                cb = b * blk
                o = st[b]["obs"]
                for t_ in range(3):
                    tl = opool.tile([128, 2, blk], F8, tag=f"op{t_}",
                                    name=f"op{t_}")
                    for i in range(2):
                        nc.sync.dma_start(tl[:, i, :], opk.ap()[t_][:, cb:cb + blk])
                    o[t_] = tl
                tl = opool.tile([128, 2, blk], F8, tag="oh", name="oht")
                for i in range(2):
                    nc.sync.dma_start(tl[:, i, :], oh.ap()[:, cb:cb + blk])
                o["oh"] = tl
                # d-pair tiles (fp8): families A (d1, kept for s4), B, C
                for fam in "ABC":
                    st[b]["d"][fam] = [
                        hpool.tile([128, 2, blk], F8, tag=f"d{fam}{c}",
                                   name=f"d{fam}{c}") for c in range(KP)
                    ]

            def accum(p, pairs):
                last = len(pairs) - 1
                for i, (l, r) in enumerate(pairs):
                    nc.tensor.matmul(p, l, r, start=(i == 0), stop=(i == last),
                                     perf_mode=DR)

            def gates_tail(b, s, f, jj, pr, pz, pgh, pg, bcp, hprev, fam):
                """Shared ACT/DVE tail; returns nothing (writes h/d/out)."""
                cb = b * blk
                c0, c1 = jj * NT, (jj + 1) * NT
                mr, mz, mn = f, 8 + f, 16 + f
                r = wpool.tile([128, NT], F16, tag="r", name="r")
                nc.scalar.activation(r[:], pr[:], AF.Sigmoid,
                                     bias=bsum_sb[:, mr:mr + 1], scale=1 / 256.)
                z = wpool.tile([128, NT], F16, tag="z", name="z")
                nc.scalar.activation(z[:], pz[:], AF.Sigmoid,
                                     bias=bsum_sb[:, mz:mz + 1], scale=1 / 256.)
                t2 = wpool.tile([128, NT], F32, tag="t2", name="t2")
                if s == 1:
                    # gh = 0: t2 = 256*(r*c_n) + pg
                    nc.vector.scalar_tensor_tensor(
                        t2[:], r[:], bhh_sb[:, mn:mn + 1], pg[:],
                        OP.mult, OP.add)
                else:
                    t1 = wpool.tile([128, NT], F32, tag="t1", name="t1")
                    nc.vector.scalar_tensor_tensor(
                        t1[:], pgh[:], bhh_sb[:, mn:mn + 1], r[:],
                        OP.add, OP.mult)
                    nc.vector.tensor_add(t2[:], t1[:], pg[:])
                n = wpool.tile([128, NT], F16, tag="n", name="n")
                nc.scalar.activation(n[:], t2[:], AF.Tanh,
                                     bias=bih_sb[:, mn:mn + 1], scale=1 / 256.)
                u = wpool.tile([128, NT], F16, tag="u", name="u")
                if s == 1:
                    nc.vector.tensor_scalar_mul(u[:], n[:], -16.0)
                else:
                    nc.vector.scalar_tensor_tensor(
                        u[:], n[:], -16.0, hprev[:], OP.mult, OP.add)
                e = wpool.tile([128, NT], F16, tag="e", name="e")
                nc.vector.tensor_mul(e[:], z[:], u[:])
                if s < 5:
                    hn = hpool.tile([128, NT], F16, tag=f"h{fam}_{f}_{jj}",
                                    name="hn")
                    nc.vector.scalar_tensor_tensor(
                        hn[:], n[:], 16.0, e[:], OP.mult, OP.add)
                    st[b]["h"][(f, jj)] = hn
                    dfam = st[b]["d"]["A" if s == 1 else "BCB"[s - 2]]
                    nc.vector.scalar_tensor_tensor(
                        dfam[f // 2][:, f % 2, c0:c1], bcp[:], 1 / 16., hn[:],
                        OP.mult, OP.add)
                else:
                    ho = wpool.tile([128, NT], F32, tag="ho", name="ho")
                    nc.vector.scalar_tensor_tensor(
                        ho[:], e[:], 1 / 16., n[:], OP.mult, OP.add)
                    nc.sync.dma_start(out_ap[f][:, cb + c0:cb + c1], ho[:])

            def step1(b):
                obs = st[b]["obs"]
                hnew = {}
                with nc.named_scope(f"b{b}s1"):
                    for f in range(FCH):
                        mr, mz, mn = f, 8 + f, 16 + f
                        for jj in range(J):
                            c0, c1 = jj * NT, (jj + 1) * NT
                            ohs = obs["oh"][:, :, c0:c1]
                            ops = obs[0][:, :, c0:c1]
                            pr = psB.tile([128, NT], F32, tag="pr", name="pr")
                            accum(pr[:], [(ts1_sb[mr][:], ohs),
                                          (wobs_sb[mr][:], ops)])
                            pz = psB.tile([128, NT], F32, tag="pz", name="pz")
                            accum(pz[:], [(ts1_sb[mz][:], ohs),
                                          (wobs_sb[mz][:], ops)])
                            pg = psB.tile([128, NT], F32, tag="pg", name="pg")
                            accum(pg[:], [(ts1_sb[mn][:], ohs),
                                          (wobs_sb[mn][:], ops)])
                            bcp = psB.tile([128, NT], F32, tag="pgh", name="bcp")
                            accum(bcp[:], [(bc_sb[0][f][:], ohs)])
                            gates_tail(b, 1, f, jj, pr, pz, None, pg, bcp,
                                       None, "A")
                st[b]["h1"] = dict(st[b]["h"])

            def stepn(b, s):
                obs = st[b]["obs"]
                hcur = st[b]["h"]
                st[b]["h"] = {}
                dprev = st[b]["d"]["ABC"[s - 2]] if s <= 4 else st[b]["d"]["B"]
                d1 = st[b]["d"]["A"]
                ot = obs[s - 1] if s <= 3 else obs[s - 3]
                wob = wobs_sb if s <= 3 else wobs45_sb
                trz = {2: t2rz_sb, 3: t3rz_sb, 4: t4rz_sb, 5: t5rz_sb}[s]
                tn = {2: t2n_sb, 3: t3n_sb, 4: t4n_sb, 5: t5n_sb}[s]
                with nc.named_scope(f"b{b}s{s}"):
                    for f in range(FCH):
                        mr, mz, mn = f, 8 + f, 16 + f
                        if s == 4:
                            vih = []
                            for m in (mr, mz, mn):
                                t = spool.tile([128, 4, 2, 128], F8, tag="wsl",
                                               name="wsl")
                                nc.sync.dma_start(t[:], wih_ap[m])
                                vih.append(t)
                        elif s == 5:
                            vs = []
                            for m in (mr, mz):
                                t = spool.tile([128, 4, 2, 128], F8, tag="wsl",
                                               name="wsl")
                                nc.sync.dma_start(t[:], wsum_ap[m])
                                vs.append(t)
                            t = spool.tile([128, 4, 2, 128], F8, tag="wsl",
                                           name="wsl")
                            nc.sync.dma_start(t[:], wih_ap[mn])
                            vs.append(t)
                        for jj in range(J):
                            c0, c1 = jj * NT, (jj + 1) * NT
                            ohs = obs["oh"][:, :, c0:c1]
                            ops = ot[:, :, c0:c1]
                            dmov = [dprev[c][:, :, c0:c1] for c in range(KP)]
                            d1mov = [d1[c][:, :, c0:c1] for c in range(KP)]

                            pr = psB.tile([128, NT], F32, tag="pr", name="pr")
                            pairs = [(trz[mr][:], ohs)]
                            if s == 5:
                                pairs += [(vs[0][:, c], dmov[c])
                                          for c in range(KP)]
                            else:
                                pairs += [(whh_sb[mr][:, c], dmov[c])
                                          for c in range(KP)]
                            if s == 4:
                                pairs += [(vih[0][:, c], d1mov[c])
                                          for c in range(KP)]
                            pairs.append((wob[mr][:], ops))
                            accum(pr[:], pairs)

                            pz = psB.tile([128, NT], F32, tag="pz", name="pz")
                            pairs = [(trz[mz][:], ohs)]
                            if s == 5:
                                pairs += [(vs[1][:, c], dmov[c])
                                          for c in range(KP)]
                            else:
                                pairs += [(whh_sb[mz][:, c], dmov[c])
                                          for c in range(KP)]
                            if s == 4:
                                pairs += [(vih[1][:, c], d1mov[c])
                                          for c in range(KP)]
                            pairs.append((wob[mz][:], ops))
                            accum(pz[:], pairs)

                            pgh = psB.tile([128, NT], F32, tag="pgh", name="pgh")
                            pairs = [(tn[f][:], ohs)]
                            pairs += [(whh_sb[mn][:, c], dmov[c])
                                      for c in range(KP)]
                            accum(pgh[:], pairs)

                            pg = psB.tile([128, NT], F32, tag="pg", name="pg")
                            if s <= 3:
                                pairs = [(ts1_sb[mn][:], ohs), (wob[mn][:], ops)]
                            elif s == 4:
                                pairs = [(t4gi_sb[f][:], ohs)]
                                pairs += [(vih[2][:, c], d1mov[c])
                                          for c in range(KP)]
                                pairs.append((wob[mn][:], ops))
                            else:
                                pairs = [(t5gi_sb[f][:], ohs)]
                                pairs += [(vs[2][:, c], dmov[c])
                                          for c in range(KP)]
                                pairs.append((wob[mn][:], ops))
                            accum(pg[:], pairs)

                            bcp = None
                            if s < 5:
                                bcp = psB.tile([128, NT], F32, tag="pg",
                                               name="bcp")
                                accum(bcp[:], [(bc_sb[s - 1][f][:], ohs)])
                            gates_tail(b, s, f, jj, pr, pz, pgh, pg, bcp,
                                       hcur[(f, jj)],
                                       {2: "B", 3: "C", 4: "B"}.get(s, ""))

            # software-pipelined block schedule
            sched = [(0, 0), (0, 1)]
            for b in range(nblk):
                sched += [(b, s) for s in (2, 3, 4)]
                if b + 1 < nblk:
                    sched += [(b + 1, 0), (b + 1, 1)]
                sched.append((b, 5))

            for b, s in sched:
                if s == 0:
                    init_block(b)
                elif s == 1:
                    step1(b)
                else:
                    stepn(b, s)

    nc.compile()
    return nc


# ---------------------------------------------------------------------------
# host-side prep / sharding
# ---------------------------------------------------------------------------

def _sig(x):
    return 1.0 / (1.0 + np.exp(-x))


def _q8(x, s=16.0):
    return (np.asarray(x, np.float32) * s).astype(E4M3)


def _2term(x, s=16.0):
    xs = np.asarray(x, np.float32) * s
    A = xs.astype(E4M3)
    B = (xs - A.astype(np.float32)).astype(E4M3)
    return A, B


def _tab_tiles(T):
    """T (nrow_ids=100, C) -> [C/128, 128, 2, 128] fp8 2-term (x16)."""
    C = T.shape[1]
    mt = C // 128
    A, B = _2term(T)
    arr = np.zeros((mt, 128, 2, 128), E4M3)
    # arr[m, p, 0, j] = A[p, 128m+j]
    arr[:, :NEMB, 0, :] = A.T.reshape(mt, 128, NEMB).transpose(0, 2, 1)
    arr[:, :NEMB, 1, :] = B.T.reshape(mt, 128, NEMB).transpose(0, 2, 1)
    return arr


def _wd_tiles(W, mt):
    """W (128mt, 1024) -> [mt, 128, 4, 2, 128] fp8: [m,p,c,i,j]=q8[128m+j,128(2c+i)+p]."""
    Q = _q8(W)
    t = Q.reshape(mt, 128, 4, 2, 128)            # [m, j, c, i, p]
    return np.ascontiguousarray(t.transpose(0, 4, 2, 3, 1))


def _wobs_tiles(Wx):
    """Wx (3072, 64) -> [24, 128, 2, 128] fp8: [[A;A]|[B;B]] packs."""
    A, B = _2term(Wx)
    arr = np.zeros((24, 128, 2, 128), E4M3)
    At = A.reshape(24, 128, 64).transpose(0, 2, 1)   # [m, p, j]
    Bt = B.reshape(24, 128, 64).transpose(0, 2, 1)
    arr[:, :64, 0, :] = At
    arr[:, 64:, 0, :] = At
    arr[:, :64, 1, :] = Bt
    arr[:, 64:, 1, :] = Bt
    return arr


def _prep_shared(emb, W_ih, W_hh, b_ih, b_hh):
    f = np.float32
    W_ih = np.asarray(W_ih, f)
    W_hh = np.asarray(W_hh, f)
    emb = np.asarray(emb, f)
    b_ih = np.asarray(b_ih, f)
    b_hh = np.asarray(b_hh, f)
    Wobs = W_ih[:, :64]
    Wemb = W_ih[:, 64:]
    Wh1 = W_ih[:, :1024]
    Wobs45 = W_ih[:, 1024:1088]
    Wsum = W_hh + Wh1
    br, bz, bn = np.split(b_ih, 3)
    cr, cz, cn = np.split(b_hh, 3)

    # mini-GRU over the 100 ids with obs=0
    giE = emb @ Wemb.T
    hb = np.zeros((NEMB, H), f)
    HBAR = []
    for t in range(3):
        gi, gh = giE, hb @ W_hh.T
        r = _sig(gi[:, :1024] + gh[:, :1024] + br + cr)
        z = _sig(gi[:, 1024:2048] + gh[:, 1024:2048] + bz + cz)
        n = np.tanh(gi[:, 2048:] + bn + r * (gh[:, 2048:] + cn))
        hb = (1 - z) * n + z * hb
        HBAR.append(hb)
    gi, gh = HBAR[0] @ Wh1.T, hb @ W_hh.T
    r = _sig(gi[:, :1024] + gh[:, :1024] + br + cr)
    z = _sig(gi[:, 1024:2048] + gh[:, 1024:2048] + bz + cz)
    n = np.tanh(gi[:, 2048:] + bn + r * (gh[:, 2048:] + cn))
    hb = (1 - z) * n + z * hb
    HBAR.append(hb)

    d = dict(
        whh=_wd_tiles(W_hh, 24),
        wih=_wd_tiles(Wh1, 24),
        wsum=_wd_tiles(Wsum[:2048], 16),
        wobs=_wobs_tiles(Wobs),
        wobs45=_wobs_tiles(Wobs45),
        ts1=_tab_tiles(giE),
        t2rz=_tab_tiles((HBAR[0] @ W_hh.T)[:, :2048] + giE[:, :2048]),
        t2n=_tab_tiles((HBAR[0] @ W_hh.T)[:, 2048:]),
        t3rz=_tab_tiles((HBAR[1] @ W_hh.T)[:, :2048] + giE[:, :2048]),
        t3n=_tab_tiles((HBAR[1] @ W_hh.T)[:, 2048:]),
        t4rz=_tab_tiles((HBAR[2] @ W_hh.T)[:, :2048] + (HBAR[0] @ Wh1.T)[:, :2048]),
        t4n=_tab_tiles((HBAR[2] @ W_hh.T)[:, 2048:]),
        t4gi=_tab_tiles((HBAR[0] @ Wh1.T)[:, 2048:]),
        t5rz=_tab_tiles((HBAR[3] @ Wsum.T)[:, :2048]),
        t5n=_tab_tiles((HBAR[3] @ W_hh.T)[:, 2048:]),
        t5gi=_tab_tiles((HBAR[3] @ Wh1.T)[:, 2048:]),
        bc=np.stack([_tab_tiles(-hbm) for hbm in HBAR]),
        bsum=np.ascontiguousarray((b_ih + b_hh).reshape(24, 128).T, f),
        bih=np.ascontiguousarray(b_ih.reshape(24, 128).T, f),
        bhh256=np.ascontiguousarray((256.0 * b_hh).reshape(24, 128).T, f),
    )
    return d


def _prep_core(buoy_obs, buoy_ids, nbuoy):
    o = np.asarray(buoy_obs, np.float32)
    ids = np.asarray(buoy_ids)
    opk = np.zeros((3, 128, nbuoy), E4M3)
    for t in range(3):
        ot = 16.0 * o[:, t, :].T                     # (64, nb)
        hi = ot.astype(E4M3)
        lo = (ot - hi.astype(np.float32)).astype(E4M3)
        opk[t, :64] = hi
        opk[t, 64:] = lo
    ohm = np.zeros((128, nbuoy), np.float32)
    ohm[ids, np.arange(nbuoy)] = 16.0
    return dict(opk=opk, oh=ohm.astype(E4M3))


_NC_CACHE = {}


def _get_nc(nbuoy, blk):
    key = (nbuoy, blk)
    if key not in _NC_CACHE:
        _NC_CACHE[key] = build(nbuoy, blk)
    return _NC_CACHE[key]


def kernel(buoy_obs, buoy_ids, emb, W_ih, W_hh, b_ih, b_hh):
    buoy_obs = np.asarray(buoy_obs)
    buoy_ids = np.asarray(buoy_ids)
    n = buoy_obs.shape[0]
    per = n // N_CORES
    shared = _prep_shared(emb, W_ih, W_hh, b_ih, b_hh)
    in_maps = []
    for c in range(N_CORES):
        sl = slice(c * per, (c + 1) * per)
        m = dict(shared)
        m.update(_prep_core(buoy_obs[sl], buoy_ids[sl], per))
        in_maps.append(m)

    nc = _get_nc(per, 1024)
    res = run_bass_kernel_spmd(nc, in_maps, list(range(N_CORES)))
    outs = []
    for c in range(N_CORES):
        r = res.results[c]["out_t"]                    # [8, 128, per]
        outs.append(r.transpose(2, 0, 1).reshape(per, H))
    full = np.concatenate(outs, axis=0).astype(np.float32)
    return full[None, :, :]
